# revision 6
# baseline (speedup 1.0000x reference)
"""GAT (2-layer, 8-head) fused Bass kernel for 8 trn2 NeuronCores.

Sharding: nodes (rows of x) split 512/core. Layer-1 h/s computed locally per
core, AllGather'd (with per-head ones + score columns piggybacked); each core
computes its 512xN attention block for all 8 heads; layer-1 output xc (+ its
layer-2 projection and scores) AllGather'd again; each core computes its
512xN layer-2 attention block and the final log_softmax rows.

Key algebra: with s_i = h_i . a_src, d_j = h_j . a_dst,
  exp(leakyrelu(s_i + d_j)) = max(exp(s_i)exp(d_j), exp(.2 s_i)exp(.2 d_j))
and softmax over j is invariant to any per-i scale, so the attention
numerator can be taken as P[i,j] = max(b_j, w_i * dd_j) with
  b_j = exp(d_j), w_i = exp(-0.8 s_i), dd_j = exp(0.2 d_j).
One DVE tensor_scalar (mult, max) per [128,512] tile; exp only on vectors.
"""

import numpy as np

N, NFEAT, NHID, NCLASS, NHEADS = 4096, 512, 64, 16, 8
NC = 8                      # cores
NQ = N // NC                # 512 own nodes per core
QT = NQ // 128              # 4 query tiles per core
JT = N // 128               # 32 key tiles
ALPHA = 0.2
HW = NHID * NHEADS          # 512
AGC = 536                   # AG payload cols: 8*(64+1) + 16 scores  (L1)
                            # L2 reuse: 512 xc + 16 outh + 1 ones + 1 sdst2 = 530 <= 536

_CACHE = {}


def _build_nc():
    import concourse.bass as bass
    import concourse.bacc as bacc
    import concourse.mybir as mybir
    import concourse.tile as tile
    from concourse.masks import make_identity

    fp32 = mybir.dt.float32
    AX = mybir.AxisListType.X
    OP = mybir.AluOpType
    AF = mybir.ActivationFunctionType

    nc = bacc.Bacc()
    xT = nc.declare_dram_parameter("xT", [NFEAT, NQ], fp32, isOutput=False)
    Whr = nc.declare_dram_parameter("Whr", [NFEAT, HW], fp32, isOutput=False)
    Asd = nc.declare_dram_parameter("Asd", [NFEAT, 16], fp32, isOutput=False)
    Wo = nc.declare_dram_parameter("Wo", [HW, NCLASS], fp32, isOutput=False)
    aod = nc.declare_dram_parameter("aod", [2, NCLASS], fp32, isOutput=False)
    out = nc.declare_dram_parameter("out", [NQ, NCLASS], fp32, isOutput=True)

    with tile.TileContext(nc) as tc:
        with (
            tc.tile_pool(name="const", bufs=1) as constp,
            tc.tile_pool(name="big", bufs=1) as bigp,
            tc.tile_pool(name="work", bufs=3) as workp,
            tc.tile_pool(name="pp", bufs=4) as ppool,
            tc.tile_pool(name="ps_acc", bufs=2, space="PSUM") as ps_acc,
            tc.tile_pool(name="ps_t", bufs=3, space="PSUM") as ps_t,
            tc.tile_pool(name="dram", bufs=1, space="DRAM") as dramp,
        ):
            v, sc, g, te, dma = nc.vector, nc.scalar, nc.gpsimd, nc.tensor, nc.sync

            ident = constp.tile([128, 128], fp32, tag="ident")
            make_identity(nc, ident[:])

            # ---- A. load params ----
            xT_sb = []
            whr_sb = []
            asd_sb = []
            wo_sb = []
            for k in range(4):
                t = constp.tile([128, NQ], fp32, tag=f"xT{k}")
                dma.dma_start(t[:], xT[k * 128:(k + 1) * 128, :])
                xT_sb.append(t)
                t = constp.tile([128, HW], fp32, tag=f"whr{k}")
                dma.dma_start(t[:], Whr[k * 128:(k + 1) * 128, :])
                whr_sb.append(t)
                t = constp.tile([128, 16], fp32, tag=f"asd{k}")
                dma.dma_start(t[:], Asd[k * 128:(k + 1) * 128, :])
                asd_sb.append(t)
                t = constp.tile([128, 16], fp32, tag=f"wo{k}")
                dma.dma_start(t[:], Wo[k * 128:(k + 1) * 128, :])
                wo_sb.append(t)
            aos_b = constp.tile([128, 16], fp32, tag="aos_b")
            dma.dma_start(aos_b[:], aod[0:1, :].to_broadcast((128, 16)))
            aod_b = constp.tile([128, 16], fp32, tag="aod_b")
            dma.dma_start(aod_b[:], aod[1:2, :].to_broadcast((128, 16)))

            ag1_in = dramp.tile([NQ, AGC], fp32, tag="ag1_in")
            ag1_out = dramp.tile([N, AGC], fp32, tag="ag1_out", addr_space="Shared")
            ag2_in = dramp.tile([NQ, AGC], fp32, tag="ag2_in")
            ag2_out = dramp.tile([N, AGC], fp32, tag="ag2_out", addr_space="Shared")
            wdram = dramp.tile([NHEADS, NQ], fp32, tag="wdram")
            w2dram = dramp.tile([1, NQ], fp32, tag="w2dram")

            # ---- B. h_ownT (feat-major) ----
            hT_sb = []
            for f in range(4):
                ps = ps_acc.tile([128, NQ], fp32, tag="acc")
                for k in range(4):
                    te.matmul(ps[:], whr_sb[k][:, f * 128:(f + 1) * 128],
                              xT_sb[k][:], start=(k == 0), stop=(k == 3))
                t = constp.tile([128, NQ], fp32, tag=f"hT{f}")
                v.tensor_copy(t[:], ps[:])
                hT_sb.append(t)

            # ---- D. s_own [16, NQ] rows 0:8 src, 8:16 dst ----
            s_ps = ps_acc.tile([16, NQ], fp32, tag="acc")
            for k in range(4):
                te.matmul(s_ps[:], asd_sb[k][:], hT_sb[k][:],
                          start=(k == 0), stop=(k == 3))
            s_sb = constp.tile([16, NQ], fp32, tag="s_sb")
            v.tensor_copy(s_sb[:], s_ps[:])

            # ---- F. w panel: exp(-0.8 * s_src) -> dram -> bcast tiles ----
            w_sb = constp.tile([NHEADS, NQ], fp32, tag="w_sb")
            sc.activation(w_sb[:], s_sb[0:NHEADS, :], AF.Exp, scale=-0.8)
            dma.dma_start(wdram[:], w_sb[:])
            wb = []
            for h in range(NHEADS):
                t = constp.tile([128, NQ], fp32, tag=f"wb{h}")
                dma.dma_start(t[:], wdram[h:h + 1, :].to_broadcast((128, NQ)))
                wb.append(t)

            # ---- C/E. h_own + stage AG1 ----
            for qt in range(QT):
                ps = ps_acc.tile([128, HW], fp32, tag="acc")
                for k in range(4):
                    te.matmul(ps[:], xT_sb[k][:, qt * 128:(qt + 1) * 128],
                              whr_sb[k][:], start=(k == 0), stop=(k == 3))
                stg = workp.tile([128, AGC], fp32, tag="stage")
                for h in range(NHEADS):
                    v.tensor_copy(stg[:, h * 65:h * 65 + 64],
                                  ps[:, h * 64:(h + 1) * 64])
                g.memset(
                    stg[:, 0:520].rearrange("p (h c) -> p h c", c=65)[:, :, 64:65],
                    1.0)
                tp = ps_t.tile([128, 16], fp32, tag="tp")
                te.transpose(tp[:], s_sb[:, qt * 128:(qt + 1) * 128], ident[0:16, 0:16])
                v.tensor_copy(stg[:, 520:536], tp[:])
                dma.dma_start(ag1_in[qt * 128:(qt + 1) * 128, :], stg[:])

            # ---- G. AllGather 1 ----
            g.collective_compute(
                "AllGather", OP.bypass,
                ins=[ag1_in.opt()], outs=[ag1_out.opt()],
                replica_groups=[list(range(NC))],
            )

            # ---- H. key-side score panels ----
            sd_pan = constp.tile([128, JT * NHEADS], fp32, tag="sd_pan")
            dma.dma_start(
                sd_pan[:].rearrange("p (t h) -> p t h", h=NHEADS),
                ag1_out[:, 528:536].rearrange("(t p) h -> p t h", p=128))
            b_all = constp.tile([128, JT * NHEADS], fp32, tag="b_all")
            sc.activation(b_all[:], sd_pan[:], AF.Exp)
            d_all = constp.tile([128, JT * NHEADS], fp32, tag="d_all")
            sc.activation(d_all[:], sd_pan[:], AF.Exp, scale=ALPHA)

            # ---- I. hx tiles (persistent keys) ----
            hx = []
            for jt in range(JT):
                t = bigp.tile([128, AGC], fp32, tag=f"hx{jt}")
                dma.dma_start(t[:], ag1_out[jt * 128:(jt + 1) * 128, :])
                hx.append(t)

            # ---- J/K. layer-1 attention ----
            xr = [bigp.tile([128, HW], fp32, tag=f"xr{qt}", name=f"xr{qt}")
                  for qt in range(QT)]
            for h in range(NHEADS):
                acc = ps_acc.tile([65, NQ], fp32, tag="acc")
                for jt in range(JT):
                    pt = ppool.tile([128, NQ], fp32, tag="pt")
                    eng = g if (jt % 3 == 2) else v
                    eng.tensor_scalar(
                        pt[:], wb[h][:],
                        d_all[:, jt * NHEADS + h:jt * NHEADS + h + 1],
                        b_all[:, jt * NHEADS + h:jt * NHEADS + h + 1],
                        OP.mult, OP.max)
                    te.matmul(acc[:], hx[jt][:, h * 65:(h + 1) * 65], pt[:],
                              start=(jt == 0), stop=(jt == JT - 1))
                fT = workp.tile([65, NQ], fp32, tag="fT")
                v.tensor_copy(fT[:], acc[:])
                for qt in range(QT):
                    tp = ps_t.tile([128, 65], fp32, tag="tp")
                    te.transpose(tp[:], fT[:, qt * 128:(qt + 1) * 128],
                                 ident[0:65, 0:65])
                    r = workp.tile([128, 1], fp32, tag="recip")
                    v.reciprocal(r[:], tp[:, 64:65])
                    v.tensor_scalar(xr[qt][:, h * 64:(h + 1) * 64],
                                    tp[:, 0:64], r[:], None, OP.mult)

            # ---- K2/L/M. elu -> xc; xcT; outh_own; stage AG2 ----
            w2tmp = constp.tile([128, QT], fp32, tag="w2tmp")
            stg2s = []
            for qt in range(QT):
                ex = workp.tile([128, HW], fp32, tag="ex")
                sc.activation(ex[:], xr[qt][:], AF.Exp)
                v.tensor_scalar(ex[:], ex[:], 1.0, 0.0, OP.subtract, OP.min)
                stg = bigp.tile([128, AGC], fp32, tag=f"stage2_{qt}")
                v.tensor_tensor(stg[:, 0:HW], xr[qt][:], ex[:], OP.max)
                stg2s.append(stg)
            xcT_sb = []
            for f in range(4):
                t = constp.tile([128, NQ], fp32, tag=f"xcT{f}")
                xcT_sb.append(t)
            for qt in range(QT):
                for f in range(4):
                    tp = ps_t.tile([128, 128], fp32, tag="tp")
                    te.transpose(tp[:], stg2s[qt][:, f * 128:(f + 1) * 128],
                                 ident[:])
                    v.tensor_copy(xcT_sb[f][:, qt * 128:(qt + 1) * 128], tp[:])
            for qt in range(QT):
                ps = ps_t.tile([128, 16], fp32, tag="tp")
                for k in range(4):
                    te.matmul(ps[:], xcT_sb[k][:, qt * 128:(qt + 1) * 128],
                              wo_sb[k][:], start=(k == 0), stop=(k == 3))
                stg = stg2s[qt]
                v.tensor_copy(stg[:, 512:528], ps[:])
                g.memset(stg[:, 528:529], 1.0)
                tmp = workp.tile([128, 16], fp32, tag="sdtmp")
                v.tensor_tensor(tmp[:], ps[:], aod_b[:], OP.mult)
                v.tensor_reduce(stg[:, 529:530], tmp[:], AX, OP.add)
                v.tensor_tensor(tmp[:], ps[:], aos_b[:], OP.mult)
                v.tensor_reduce(w2tmp[:, qt:qt + 1], tmp[:], AX, OP.add)
                g.memset(stg[:, 530:536], 0.0)
                dma.dma_start(ag2_in[qt * 128:(qt + 1) * 128, :], stg[:])

            # ---- N. w2 bcast ----
            w2e = constp.tile([128, QT], fp32, tag="w2e")
            sc.activation(w2e[:], w2tmp[:], AF.Exp, scale=-0.8)
            dma.dma_start(w2dram[:].rearrange("o (t p) -> p (o t)", p=128), w2e[:])
            w2b = constp.tile([128, NQ], fp32, tag="w2b")
            dma.dma_start(w2b[:], w2dram[0:1, :].to_broadcast((128, NQ)))

            # ---- O. AllGather 2 ----
            g.collective_compute(
                "AllGather", OP.bypass,
                ins=[ag2_in.opt()], outs=[ag2_out.opt()],
                replica_groups=[list(range(NC))],
            )

            # ---- P. layer-2 panels ----
            hx2 = constp.tile([128, JT * 17], fp32, tag="hx2")
            dma.dma_start(
                hx2[:].rearrange("p (t c) -> p t c", c=17),
                ag2_out[:, 512:529].rearrange("(t p) c -> p t c", p=128))
            sd2 = constp.tile([128, JT], fp32, tag="sd2")
            dma.dma_start(
                sd2[:].rearrange("p (t c) -> p t c", c=1),
                ag2_out[:, 529:530].rearrange("(t p) c -> p t c", p=128))
            b2 = constp.tile([128, JT], fp32, tag="b2")
            sc.activation(b2[:], sd2[:], AF.Exp)
            d2 = constp.tile([128, JT], fp32, tag="d2")
            sc.activation(d2[:], sd2[:], AF.Exp, scale=ALPHA)

            # ---- Q. layer-2 attention ----
            acc2 = ps_acc.tile([17, NQ], fp32, tag="acc")
            for jt in range(JT):
                pt = ppool.tile([128, NQ], fp32, tag="pt")
                eng = g if (jt % 3 == 2) else v
                eng.tensor_scalar(pt[:], w2b[:],
                                  d2[:, jt:jt + 1], b2[:, jt:jt + 1],
                                  OP.mult, OP.max)
                te.matmul(acc2[:], hx2[:, jt * 17:(jt + 1) * 17], pt[:],
                          start=(jt == 0), stop=(jt == JT - 1))
            f2 = workp.tile([17, NQ], fp32, tag="f2")
            v.tensor_copy(f2[:], acc2[:])

            # ---- R. normalize, elu, log_softmax, store ----
            for qt in range(QT):
                tp = ps_t.tile([128, 17], fp32, tag="tp")
                te.transpose(tp[:], f2[:, qt * 128:(qt + 1) * 128],
                             ident[0:17, 0:17])
                r = workp.tile([128, 1], fp32, tag="recip")
                v.reciprocal(r[:], tp[:, 16:17])
                o = workp.tile([128, NCLASS], fp32, tag="o")
                v.tensor_scalar(o[:], tp[:, 0:16], r[:], None, OP.mult)
                eo = workp.tile([128, NCLASS], fp32, tag="eo")
                sc.activation(eo[:], o[:], AF.Exp)
                v.tensor_scalar(eo[:], eo[:], 1.0, 0.0, OP.subtract, OP.min)
                elu = workp.tile([128, NCLASS], fp32, tag="elu")
                v.tensor_tensor(elu[:], o[:], eo[:], OP.max)
                se = workp.tile([128, 1], fp32, tag="se")
                e2 = workp.tile([128, NCLASS], fp32, tag="e2")
                sc.activation(e2[:], elu[:], AF.Exp, accum_out=se[:])
                lse = workp.tile([128, 1], fp32, tag="lse")
                sc.activation(lse[:], se[:], AF.Ln)
                fin = workp.tile([128, NCLASS], fp32, tag="fin")
                v.tensor_scalar(fin[:], elu[:], lse[:], None, OP.subtract)
                dma.dma_start(out[qt * 128:(qt + 1) * 128, :], fin[:])

    nc.finalize()
    return nc


def _get_compiled():
    if "nc" not in _CACHE:
        _CACHE["nc"] = _build_nc()
    return _CACHE["nc"]


def kernel(x, Wh, ah, Wo, ao):
    from concourse.bass_utils import run_bass_kernel_spmd

    nc = _get_compiled()
    x = np.asarray(x, np.float32)
    Wh = np.asarray(Wh, np.float32)
    ah = np.asarray(ah, np.float32)
    Wo = np.asarray(Wo, np.float32)
    ao = np.asarray(ao, np.float32)

    # host-side relayouts (no math): head-major weight matrix, block-diag
    # score matrix, split ao
    Whr = np.ascontiguousarray(
        Wh.transpose(1, 0, 2).reshape(NFEAT, HW))          # [512, 512]
    Asd = np.zeros((NFEAT, 16), np.float32)
    for h in range(NHEADS):
        Asd[h * NHID:(h + 1) * NHID, h] = ah[h, :NHID]      # src
        Asd[h * NHID:(h + 1) * NHID, 8 + h] = ah[h, NHID:]  # dst
    aod = np.stack([ao[:NCLASS], ao[NCLASS:]])              # [2, 16]

    in_maps = []
    for i in range(NC):
        in_maps.append({
            "xT": np.ascontiguousarray(x[i * NQ:(i + 1) * NQ].T),
            "Whr": Whr, "Asd": Asd,
            "Wo": np.ascontiguousarray(Wo), "aod": aod,
        })
    res = run_bass_kernel_spmd(nc, in_maps, list(range(NC)))
    return np.concatenate([res.results[i]["out"] for i in range(NC)], 0)
